# revision 1
# baseline (speedup 1.0000x reference)
"""TRN2 Bass kernel for sliding-window causal GQA attention block.

Reference computation (B=2, T=2048, C=2048, NH=16, NKV=4, HD=128, WIN=512):
  qkv = x @ w_qkv.T ; RoPE(q, k) ; GQA repeat ; banded causal attention
  (keys j in [i-511, i]) ; y @ w_proj.T

Sharding: 8 cores = batch (2) x kv-head-group (4) tensor parallel.
Core c = b*4+g owns batch b, q-heads [4g..4g+4), kv head g. Each core
computes a partial output (contribution of its 512 y-dims to all 2048
out dims); host sums the 4 partials per batch.

Everything on-chip is kept "transposed" ([feature, token]) so that all
matmuls have their contraction on the partition axis without any
on-chip layout changes except small PE transposes for probs/v/y.
"""
import sys
sys.path.insert(0, '/opt/trn_rl_repo')
import numpy as np
import ml_dtypes

import concourse.bass as bass
from concourse import bacc
import concourse.tile as tile
from concourse import mybir
from concourse.bass_utils import run_bass_kernel_spmd
from concourse.masks import make_identity

T = 2048
C = 2048
HD = 128
NH = 16
NKV = 4
NQL = 4           # q heads per core
WIN = 512
QKVF = NQL * HD + 2 * HD   # 768 local qkv features
SCALE = float(1.0 / np.sqrt(HD))
QB = T // 128     # 16 q blocks
KC = C // 128     # 16 contraction tiles
NEG = -1e9

f32 = mybir.dt.float32
f32r = mybir.dt.float32r
bf16 = mybir.dt.bfloat16

_CACHE = {}


def _build_program():
    nc = bacc.Bacc()
    xT = nc.declare_dram_parameter("xT", [C, T], f32r, isOutput=False)
    wqkvT = nc.declare_dram_parameter("wqkvT", [C, QKVF], f32r, isOutput=False)
    wpT = nc.declare_dram_parameter("wpT", [NQL * HD, C], bf16, isOutput=False)
    cosE = nc.declare_dram_parameter("cosE", [HD, T], f32, isOutput=False)
    sinE = nc.declare_dram_parameter("sinE", [HD, T], f32, isOutput=False)
    mdiag = nc.declare_dram_parameter("mdiag", [128, 128], f32, isOutput=False)
    mwin = nc.declare_dram_parameter("mwin", [128, 128], f32, isOutput=False)
    rotP = nc.declare_dram_parameter("rotP", [128, 128], f32r, isOutput=False)
    outT = nc.declare_dram_parameter("outT", [C, T], f32, isOutput=True)

    import os as _os
    _tsim = _os.environ.get("KERNEL_TRACE_SIM", "0") == "1"
    with tile.TileContext(nc, trace_sim=_tsim) as tc:
        with tc.tile_pool(name="persist", bufs=1) as persist, \
             tc.tile_pool(name="qkv_out", bufs=1) as qkv_out, \
             tc.tile_pool(name="ytile", bufs=1) as ytile, \
             tc.tile_pool(name="outst", bufs=3) as outst, \
             tc.tile_pool(name="wq", bufs=1) as wqp, \
             tc.tile_pool(name="xs", bufs=18) as xsp, \
             tc.tile_pool(name="rope_tmp", bufs=2) as rtp, \
             tc.tile_pool(name="attn_sb", bufs=6) as asb:

            # ---- persistent small tensors (ACT HWDGE ring: off the
            # SP ring that streams weights/activations) ----
            cos_sb = persist.tile([HD, T], f32, tag="cos")
            sin_sb = persist.tile([HD, T], f32, tag="sin")
            nc.scalar.dma_start(out=cos_sb, in_=cosE[:])
            nc.scalar.dma_start(out=sin_sb, in_=sinE[:])
            md_sb = persist.tile([128, 128], f32, tag="md")
            mw_sb = persist.tile([128, 128], f32, tag="mw")
            nc.scalar.dma_start(out=md_sb, in_=mdiag[:])
            nc.scalar.dma_start(out=mw_sb, in_=mwin[:])
            rp_sb = persist.tile([128, 128], f32r, tag="rp")
            nc.scalar.dma_start(out=rp_sb, in_=rotP[:])
            ident_b = persist.tile([128, 128], bf16, tag="idb")
            make_identity(nc, ident_b)
            wp_sb = persist.tile([128, NQL, C], bf16, tag="wp")
            nc.scalar.dma_start(out=wp_sb, in_=wpT[:].rearrange("(kd p) o -> p kd o", p=128))

            # qkv outputs (transposed layout [feature, token])
            qT = [qkv_out.tile([HD, T], f32r, tag=f"qT{h}", name=f"qT{h}")
                  for h in range(NQL)]
            kT = qkv_out.tile([HD, T], f32r, tag="kT")
            vTb = qkv_out.tile([HD, T], bf16, tag="vTb")
            v_sb = [qkv_out.tile([128, HD], bf16, tag=f"v{t}", name=f"v{t}")
                    for t in range(QB)]
            yT = [ytile.tile([HD, T], bf16, tag=f"yT{h}", name=f"yT{h}")
                  for h in range(NQL)]

            # interleaved weight + first-chunk activation loads: the first
            # accumulation is DMA-arrival-paced
            w_tiles = []
            xt0 = []
            for k in range(KC):
                w_k = wqp.tile([128, QKVF], f32r, tag=f"w{k}", name=f"w{k}")
                nc.sync.dma_start(out=w_k, in_=wqkvT[k * 128:(k + 1) * 128, :])
                w_tiles.append(w_k)
                x_0k = xsp.tile([128, 512], f32r, tag="x", name=f"x0{k}")
                nc.sync.dma_start(out=x_0k, in_=xT[k * 128:(k + 1) * 128, 0:512])
                xt0.append(x_0k)

            def attn_block(h, qb, scps, ypps, ptps):
                kt_lo = max(0, qb - 4)
                nk = qb - kt_lo + 1
                w = nk * 128
                sc = scps.tile([128, 640], f32, tag="sc", name=f"sc{h}_{qb}")
                lhs_q = qT[h][:, qb * 128:(qb + 1) * 128]
                for c0 in range(0, w, 512):
                    cw = min(512, w - c0)
                    nc.tensor.matmul(sc[:, c0:c0 + cw], lhs_q,
                                     kT[:, kt_lo * 128 + c0: kt_lo * 128 + c0 + cw],
                                     start=True, stop=True)
                # causal mask on the diagonal tile, window mask on the leftmost
                nc.vector.tensor_add(out=sc[:, w - 128:w], in0=sc[:, w - 128:w], in1=md_sb)
                if qb >= 4:
                    nc.vector.tensor_add(out=sc[:, 0:128], in0=sc[:, 0:128], in1=mw_sb)
                probs = asb.tile([128, 640], bf16, tag="probs", name=f"pr{h}_{qb}")
                ssum = asb.tile([128, 1], f32, tag="ssum", name=f"ss{h}_{qb}")
                nc.scalar.activation(out=probs[:, :w], in_=sc[:, :w],
                                     func=mybir.ActivationFunctionType.Exp,
                                     scale=SCALE, accum_out=ssum)
                yp = ypps.tile([128, HD], f32, tag="yp", name=f"yp{h}_{qb}")
                for j in range(nk):
                    pt = ptps.tile([128, 128], bf16, tag="pt", name=f"ptp{h}_{qb}_{j}")
                    nc.tensor.transpose(pt, probs[:, j * 128:(j + 1) * 128], ident_b)
                    pts = asb.tile([128, 128], bf16, tag="pts", name=f"pt{h}_{qb}_{j}")
                    if j % 2 == 0:
                        nc.vector.tensor_copy(out=pts, in_=pt)
                    else:
                        nc.scalar.copy(out=pts, in_=pt)
                    nc.tensor.matmul(yp, pts, v_sb[kt_lo + j],
                                     start=(j == 0), stop=(j == nk - 1))
                rr = asb.tile([128, 1], f32, tag="rr", name=f"rr{h}_{qb}")
                nc.vector.reciprocal(rr, ssum)
                yb = asb.tile([128, HD], bf16, tag="yb", name=f"yb{h}_{qb}")
                nc.vector.tensor_scalar_mul(yb, yp, rr)
                ytp = ptps.tile([128, 128], bf16, tag="pt", name=f"ytp{h}_{qb}")
                nc.tensor.transpose(ytp, yb, ident_b)
                nc.scalar.copy(out=yT[h][:, qb * 128:(qb + 1) * 128], in_=ytp)

            def proj_chunk(n, pjps):
                for mo in range(C // 128):
                    pp = pjps.tile([128, 512], f32, tag="pp", name=f"pp{mo}_{n}")
                    for kd in range(NQL):
                        nc.tensor.matmul(pp, wp_sb[:, kd, mo * 128:(mo + 1) * 128],
                                         yT[kd][:, n * 512:(n + 1) * 512],
                                         start=(kd == 0), stop=(kd == NQL - 1))
                    os_t = outst.tile([128, 512], f32, tag="os", name=f"os{mo}_{n}")
                    nc.vector.tensor_copy(out=os_t, in_=pp)
                    nc.sync.dma_start(out=outT[mo * 128:(mo + 1) * 128, n * 512:(n + 1) * 512],
                                      in_=os_t)

            # ---- phase 1: QKV + rope + v-transpose, chunk by chunk ----
            with tc.tile_pool(name="qkps", bufs=3, space="PSUM") as qkps, \
                 tc.tile_pool(name="ropeps", bufs=2, space="PSUM") as rops:
                for n in range(T // 512):
                    ns = slice(n * 512, (n + 1) * 512)
                    if n == 0:
                        xt = xt0
                    else:
                        xt = []
                        for k in range(KC):
                            x_nk = xsp.tile([128, 512], f32r, tag="x")
                            nc.sync.dma_start(out=x_nk, in_=xT[k * 128:(k + 1) * 128, n * 512:(n + 1) * 512])
                            xt.append(x_nk)
                    # qkv for this 512-token chunk
                    for m in range(QKVF // 128):
                        acc = qkps.tile([128, 512], f32, tag="acc", name=f"acc{n}_{m}")
                        for k in range(KC):
                            nc.tensor.matmul(acc, w_tiles[k][:, m * 128:(m + 1) * 128],
                                             xt[k],
                                             start=(k == 0), stop=(k == KC - 1))
                        if m < NQL:
                            nc.scalar.copy(out=qT[m][:, ns], in_=acc)
                        elif m == NQL:
                            nc.scalar.copy(out=kT[:, ns], in_=acc)
                        else:
                            nc.scalar.copy(out=vTb[:, ns], in_=acc)
                    # rope this chunk (PE rotate via signed permutation
                    # matrix; DVE cannot cross partitions)
                    for th in range(NQL + 1):
                        src = qT[th] if th < NQL else kT
                        rot = rops.tile([HD, 512], f32, tag="rot", name=f"rot{n}_{th}")
                        nc.tensor.matmul(rot, rp_sb, src[:, ns], start=True, stop=True)
                        tmp = rtp.tile([HD, 512], f32, tag="tmp")
                        nc.vector.tensor_mul(out=tmp, in0=rot, in1=sin_sb[:, ns])
                        nc.vector.tensor_mul(out=src[:, ns], in0=src[:, ns], in1=cos_sb[:, ns])
                        nc.vector.tensor_add(out=src[:, ns], in0=src[:, ns], in1=tmp)
                    # v transpose (xbar DMA transpose, ACT ring)
                    for t in range(4 * n, 4 * n + 4):
                        nc.scalar.dma_start_transpose(v_sb[t], vTb[:, t * 128:(t + 1) * 128])

            # ---- phase 2: attention (qb-major) + per-chunk projection ----
            with tc.tile_pool(name="scps", bufs=2, space="PSUM") as scps, \
                 tc.tile_pool(name="ptps", bufs=2, space="PSUM") as ptps, \
                 tc.tile_pool(name="ypps", bufs=2, space="PSUM") as ypps:
                for qb in range(QB):
                    for h in range(NQL):
                        attn_block(h, qb, scps, ypps, ptps)
            with tc.tile_pool(name="pjps", bufs=3, space="PSUM") as pjps:
                for n in range(T // 512):
                    proj_chunk(n, pjps)
    nc.finalize()
    return nc


def _prep_inputs(x, w_qkv, w_proj, freqs_cos, freqs_sin):
    """Build the 8 per-core input maps (host-side shard + transpose)."""
    x = np.asarray(x, dtype=np.float32)
    w_qkv = np.asarray(w_qkv, dtype=np.float32)
    w_proj = np.asarray(w_proj, dtype=np.float32)
    freqs_cos = np.asarray(freqs_cos, dtype=np.float32)
    freqs_sin = np.asarray(freqs_sin, dtype=np.float32)

    # interleaved-pair rope tables expanded to [HD, T]
    cosE = np.ascontiguousarray(np.repeat(freqs_cos.T, 2, axis=0))
    sinE = np.ascontiguousarray(np.repeat(freqs_sin.T, 2, axis=0))
    # signed pair-rotation matrix: rot = P.T @ t, rot[2r] = -t[2r+1], rot[2r+1] = t[2r]
    rotP = np.zeros((HD, HD), np.float32)
    idx = np.arange(0, HD, 2)
    rotP[idx + 1, idx] = -1.0
    rotP[idx, idx + 1] = 1.0
    r = np.arange(128)[:, None]
    jj = np.arange(128)[None, :]
    mdiag = np.where(jj <= r, 0.0, NEG).astype(np.float32)
    mwin = np.where(jj > r, 0.0, NEG).astype(np.float32)

    xTs = [np.ascontiguousarray(x[b].T) for b in range(2)]
    in_maps = []
    for c in range(8):
        b, g = divmod(c, 4)
        wq = w_qkv[g * NQL * HD:(g + 1) * NQL * HD]          # [512, C]
        wk = w_qkv[NH * HD + g * HD: NH * HD + (g + 1) * HD]  # [128, C]
        wv = w_qkv[(NH + NKV) * HD + g * HD: (NH + NKV) * HD + (g + 1) * HD]
        wqkvT = np.ascontiguousarray(np.concatenate([wq, wk, wv], axis=0).T)
        wpT = np.ascontiguousarray(
            w_proj[:, g * NQL * HD:(g + 1) * NQL * HD].T).astype(ml_dtypes.bfloat16)
        in_maps.append({
            "xT": xTs[b], "wqkvT": wqkvT, "wpT": wpT,
            "cosE": cosE, "sinE": sinE, "mdiag": mdiag, "mwin": mwin,
            "rotP": rotP,
        })
    return in_maps


def _run(in_maps, trace=False):
    if "nc" not in _CACHE:
        _CACHE["nc"] = _build_program()
    return run_bass_kernel_spmd(_CACHE["nc"], in_maps, core_ids=list(range(8)),
                                trace=False)


def kernel(x, w_qkv, w_proj, freqs_cos, freqs_sin, mask=None, _trace=False):
    in_maps = _prep_inputs(x, w_qkv, w_proj, freqs_cos, freqs_sin)
    res = _run(in_maps, trace=_trace)
    out = np.empty((2, T, C), dtype=np.float32)
    for b in range(2):
        acc = res.results[b * 4]["outT"].astype(np.float32)
        for g in range(1, 4):
            acc = acc + res.results[b * 4 + g]["outT"]
        out[b] = acc.T
    if _trace:
        return out, res
    return out



# revision 2
# speedup vs baseline: 2.8724x; 2.8724x over previous
"""TRN2 Bass kernel for sliding-window causal GQA attention block (v4).

Reference computation (B=2, T=2048, C=2048, NH=16, NKV=4, HD=128, WIN=512):
  qkv = x @ w_qkv.T ; RoPE(q, k) ; GQA repeat ; banded causal attention
  (keys j in [i-511, i]) ; y @ w_proj.T

Sharding: 8 cores = batch (2) x kv-head-group (4) tensor parallel.
Core c = b*4+g owns batch b, q-heads [4g..4g+4), kv head g. Each core
computes a partial output (contribution of its 512 y-dims to all 2048
out dims); host sums the 4 partials per batch.

Key design points (v4):
- All PE work in bf16 (FWL weight loads, full-rate matmuls).
- Attention scores are computed TRANSPOSED ([key, query]) per key
  tile, so exp'd probs feed the P@V matmul directly as the moving
  operand -- no PE transposes of probs.  Causal / window masks are 0/1
  multiplies on the probs (off the critical chain).
- P@V and the softmax denominator accumulate per (head, 512-token
  q chunk) with per-element has_written semantics: one matmul per
  contributing key tile, each writing just its valid column span.
- Denominators come from ones-vector matmuls; normalization of y^T is
  one reciprocal + ones-outer-product broadcast multiply per chunk.
- DMA op count is minimized (the HWDGE FIFOs serialize and each op
  carries ~2us of fixed cost): one aux-constant load, one wqkv load,
  one wproj load, 4 x-chunk loads, 4 output stores.  V tiles are
  transposed on the PE (identity matmul), not via xbar DMA.

``_build_program(reps=N)`` unrolls the whole body N times (fresh tile
pools per rep, all DMAs re-issued) -- used by the bench to measure
per-execution device time free of dispatch overhead.
"""
import sys
sys.path.insert(0, '/opt/trn_rl_repo')
import numpy as np
import ml_dtypes

import concourse.bass as bass
from concourse import bacc
import concourse.tile as tile
from concourse import mybir
from concourse.bass_utils import run_bass_kernel_spmd

T = 2048
C = 2048
HD = 128
NH = 16
NKV = 4
NQL = 4           # q heads per core
WIN = 512
QKVF = NQL * HD + 2 * HD   # 768 local qkv features
SCALE = float(1.0 / np.sqrt(HD))
QB = T // 128     # 16 q/k blocks
KC = 16           # contraction tiles (C/128)
NCH = T // 512    # 4 token chunks

# aux constant-pack column offsets
COS_OFF = 0
SIN_OFF = 2048
ROT_OFF = 4096
MA_OFF = 4224
MB_OFF = 4352
ID_OFF = 4480
ONE_OFF = 4608
AUXW = 4608 + 130

f32 = mybir.dt.float32
bf16 = mybir.dt.bfloat16

_CACHE = {}


def _emit_body(nc, tc, dram, rep):
    """One full forward pass. dram: dict of DRAM parameter APs.

    KERNEL_PHASES env (debug/bench only): "1" = qkv only, "12" = +attn,
    "123" (default) = full. Disabled phases are replaced by a liveness
    store so DCE cannot remove the remaining work.
    """
    import os as _os
    _phases = _os.environ.get("KERNEL_PHASES", "123")
    r = rep

    with tc.tile_pool(name=f"persist{r}", bufs=1) as persist, \
         tc.tile_pool(name=f"qkv_out{r}", bufs=1) as qkv_out:

        # ---- constants: one packed load on the ACT ring, wproj on the
        # ACT ring, wqkv on the gpsimd (SWDGE) ring so all three start
        # in parallel with the first x chunk on the SP ring ----
        aux = persist.tile([128, AUXW], bf16, tag="aux")
        nc.scalar.dma_start(out=aux, in_=dram["aux"][:])
        cos_sb = aux[:, COS_OFF:COS_OFF + T]
        sin_sb = aux[:, SIN_OFF:SIN_OFF + T]
        rp_sb = aux[:, ROT_OFF:ROT_OFF + 128]
        mA_sb = aux[:, MA_OFF:MA_OFF + 128]
        mB_sb = aux[:, MB_OFF:MB_OFF + 128]
        id_sb = aux[:, ID_OFF:ID_OFF + 128]
        o1_sb = aux[:, ONE_OFF:ONE_OFF + 1]
        oc_sb = aux[0:1, ONE_OFF + 1:ONE_OFF + 129]
        wp_sb = persist.tile([128, NQL, C], bf16, tag="wp")
        nc.scalar.dma_start(out=wp_sb, in_=dram["wpT"][:].rearrange("(kd p) o -> p kd o", p=128))
        w_sb = persist.tile([128, KC, QKVF], bf16, tag="w")
        nc.gpsimd.dma_start(out=w_sb, in_=dram["wqkvT"][:].rearrange("(k p) f -> p k f", p=128))

        # qkv outputs (transposed layout [feature, token], bf16)
        qT = [qkv_out.tile([HD, T], bf16, tag=f"qT{h}", name=f"qT{r}_{h}")
              for h in range(NQL)]
        kT = qkv_out.tile([HD, T], bf16, tag="kT", name=f"kT{r}")
        v_sb = [qkv_out.tile([128, HD], bf16, tag=f"v{t}", name=f"v{r}_{t}")
                for t in range(QB)]
        yT = [qkv_out.tile([HD, T], bf16, tag=f"yT{h}", name=f"yT{r}_{h}")
              for h in range(NQL)]
        rr_sb = [qkv_out.tile([1, T], bf16, tag=f"rr{h}", name=f"rr{r}_{h}")
                 for h in range(NQL)]

        # ---- phase 1: QKV + rope + v-transpose, chunk by chunk ----
        with tc.tile_pool(name=f"xs{r}", bufs=2) as xsp, \
             tc.tile_pool(name=f"vtb{r}", bufs=1) as vtbp, \
             tc.tile_pool(name=f"rtp{r}", bufs=2) as rtp, \
             tc.tile_pool(name=f"qkps{r}", bufs=3, space="PSUM") as qkps, \
             tc.tile_pool(name=f"rops{r}", bufs=2, space="PSUM") as rops, \
             tc.tile_pool(name=f"vtps{r}", bufs=2, space="PSUM") as vtps:
            vTb = vtbp.tile([HD, T], bf16, tag="vTb", name=f"vTb{r}")
            xt = []
            for n in range(NCH):
                x_n = xsp.tile([128, KC, 512], bf16, tag="x", name=f"x{r}_{n}")
                nc.sync.dma_start(
                    out=x_n,
                    in_=dram["xT"][:, n * 512:(n + 1) * 512].rearrange("(k p) t -> p k t", p=128))
                xt.append(x_n)

            for n in range(NCH):
                ns = slice(n * 512, (n + 1) * 512)
                # qkv for this 512-token chunk
                for m in range(QKVF // 128):
                    acc = qkps.tile([128, 512], f32, tag="acc", name=f"acc{r}_{n}_{m}")
                    for k in range(KC):
                        nc.tensor.matmul(acc, w_sb[:, k, m * 128:(m + 1) * 128],
                                         xt[n][:, k, :],
                                         start=(k == 0), stop=(k == KC - 1))
                    if m < NQL:
                        nc.vector.tensor_copy(out=qT[m][:, ns], in_=acc)
                    elif m == NQL:
                        nc.vector.tensor_copy(out=kT[:, ns], in_=acc)
                    else:
                        nc.vector.tensor_copy(out=vTb[:, ns], in_=acc)
                # rope this chunk (PE rotate via signed permutation
                # matrix; DVE cannot cross partitions)
                for th in range(NQL + 1):
                    src = qT[th] if th < NQL else kT
                    rot = rops.tile([HD, 512], f32, tag="rot", name=f"rot{r}_{n}_{th}")
                    nc.tensor.matmul(rot, rp_sb, src[:, ns], start=True, stop=True)
                    tmp = rtp.tile([HD, 512], bf16, tag="tmp")
                    nc.vector.tensor_mul(out=tmp, in0=rot, in1=sin_sb[:, ns])
                    nc.vector.tensor_mul(out=src[:, ns], in0=src[:, ns], in1=cos_sb[:, ns])
                    nc.vector.tensor_add(out=src[:, ns], in0=src[:, ns], in1=tmp)
                # v transpose on the PE (identity matmul)
                for t in range(4 * n, 4 * n + 4):
                    vt = vtps.tile([128, 128], bf16, tag="vt", name=f"vt{r}_{t}")
                    nc.tensor.transpose(vt, vTb[:, t * 128:(t + 1) * 128], id_sb)
                    nc.vector.tensor_copy(out=v_sb[t], in_=vt)

        if "2" not in _phases:
            # liveness: store q/k so phase 1 cannot be DCE'd
            for h in range(NQL):
                nc.sync.dma_start(out=dram["outT"][h * 128:(h + 1) * 128, :], in_=qT[h])
            nc.sync.dma_start(out=dram["outT"][512:640, :], in_=kT)
            return

        # ---- phase 2: attention, key-tile-major transposed scores;
        # P@V + denominator accumulate per (head, 512-token q chunk),
        # consumed one chunk behind the score production ----
        probs_t = [[None] * QB for _ in range(NQL)]

        def consume_chunk(c, ypps, dnps):
            jlo = max(0, 4 * c - 4)
            jhi = 4 * c + 3
            for h in range(NQL):
                yp = ypps.tile([HD, 512], f32, tag="yp", name=f"yp{r}_{c}_{h}")
                dn = dnps.tile([1, 512], f32, tag="dn", name=f"dn{r}_{c}_{h}")
                for j in range(jlo, jhi + 1):
                    qlo = max(4 * c, j)
                    qhi = min(4 * c + 3, j + 4)
                    sl = probs_t[h][j][:, (qlo - j) * 128:(qhi - j + 1) * 128]
                    cs = slice((qlo - 4 * c) * 128, (qhi - 4 * c + 1) * 128)
                    nc.tensor.matmul(yp[:, cs], v_sb[j], sl,
                                     start=(j == jlo), stop=(j == jhi))
                    nc.tensor.matmul(dn[:, cs], o1_sb, sl,
                                     start=(j == jlo), stop=(j == jhi))
                with nc.allow_low_precision(reason="bf16 reciprocal row; 0.4% ok"):
                    nc.vector.reciprocal(rr_sb[h][:, c * 512:(c + 1) * 512], dn)
                nc.vector.tensor_copy(out=yT[h][:, c * 512:(c + 1) * 512], in_=yp)

        with tc.tile_pool(name=f"prb{r}", bufs=10) as prb, \
             tc.tile_pool(name=f"stps{r}", bufs=2, space="PSUM") as stps, \
             tc.tile_pool(name=f"ypps{r}", bufs=2, space="PSUM") as ypps, \
             tc.tile_pool(name=f"dnps{r}", bufs=2, space="PSUM") as dnps:
            for j in range(QB):
                w = 128 * min(5, QB - j)
                for h in range(NQL):
                    # scores^T for key tile j against q blocks j..j+4
                    st = stps.tile([128, 640], f32, tag="st", name=f"st{r}_{j}_{h}")
                    for c0 in range(0, w, 512):
                        cw = min(512, w - c0)
                        nc.tensor.matmul(st[:, c0:c0 + cw],
                                         kT[:, j * 128:(j + 1) * 128],
                                         qT[h][:, j * 128 + c0: j * 128 + c0 + cw],
                                         start=True, stop=True)
                    pb = prb.tile([128, 640], bf16, tag=f"pb{h}", name=f"pb{r}_{j}_{h}")
                    nc.scalar.activation(out=pb[:, :w], in_=st[:, :w],
                                         func=mybir.ActivationFunctionType.Exp,
                                         scale=SCALE)
                    # multiplicative 0/1 masks after exp: diagonal tile
                    # (cols 0:128, keep k<=q) and window-edge tile
                    # (cols 512:640, keep k>q)
                    nc.vector.tensor_mul(out=pb[:, 0:128], in0=pb[:, 0:128], in1=mA_sb)
                    if w == 640:
                        nc.vector.tensor_mul(out=pb[:, 512:640], in0=pb[:, 512:640], in1=mB_sb)
                    probs_t[h][j] = pb
                if j in (4, 8, 12):
                    consume_chunk(j // 4 - 1, ypps, dnps)
            consume_chunk(3, ypps, dnps)

        if "3" not in _phases:
            for h in range(NQL):
                nc.sync.dma_start(out=dram["outT"][h * 128:(h + 1) * 128, :], in_=yT[h])
                nc.sync.dma_start(out=dram["outT"][512 + h:513 + h, :], in_=rr_sb[h])
            return

        # ---- phase 3: normalize y^T per chunk + projection ----
        with tc.tile_pool(name=f"outst{r}", bufs=2) as outst, \
             tc.tile_pool(name=f"pjps{r}", bufs=3, space="PSUM") as pjps, \
             tc.tile_pool(name=f"bcps{r}", bufs=2, space="PSUM") as bcps:
            for n in range(NCH):
                ns = slice(n * 512, (n + 1) * 512)
                for h in range(NQL):
                    bc = bcps.tile([128, 512], f32, tag="bc", name=f"bc{r}_{n}_{h}")
                    nc.tensor.matmul(bc, oc_sb, rr_sb[h][:, ns], start=True, stop=True)
                    nc.vector.tensor_mul(out=yT[h][:, ns], in0=yT[h][:, ns], in1=bc)
                os_big = outst.tile([128, C // 128, 512], bf16, tag="os", name=f"os{r}_{n}")
                for mo in range(C // 128):
                    pp = pjps.tile([128, 512], f32, tag="pp", name=f"pp{r}_{mo}_{n}")
                    for kd in range(NQL):
                        nc.tensor.matmul(pp, wp_sb[:, kd, mo * 128:(mo + 1) * 128],
                                         yT[kd][:, ns],
                                         start=(kd == 0), stop=(kd == NQL - 1))
                    nc.vector.tensor_copy(out=os_big[:, mo, :], in_=pp)
                nc.sync.dma_start(
                    out=dram["outT"][:, n * 512:(n + 1) * 512].rearrange("(mo p) t -> p mo t", p=128),
                    in_=os_big)


def _build_program(reps=1):
    nc = bacc.Bacc()
    dram = {
        "xT": nc.declare_dram_parameter("xT", [C, T], bf16, isOutput=False),
        "wqkvT": nc.declare_dram_parameter("wqkvT", [C, QKVF], bf16, isOutput=False),
        "wpT": nc.declare_dram_parameter("wpT", [NQL * HD, C], bf16, isOutput=False),
        "aux": nc.declare_dram_parameter("aux", [128, AUXW], bf16, isOutput=False),
        "outT": nc.declare_dram_parameter("outT", [C, T], bf16, isOutput=True),
    }

    import os as _os
    _tsim = _os.environ.get("KERNEL_TRACE_SIM", "0") == "1"
    with tile.TileContext(nc, trace_sim=_tsim) as tc:
        for rep in range(reps):
            _emit_body(nc, tc, dram, rep)
    nc.finalize()
    return nc


def _prep_inputs(x, w_qkv, w_proj, freqs_cos, freqs_sin):
    """Build the 8 per-core input maps (host-side shard + transpose)."""
    x = np.asarray(x, dtype=np.float32)
    w_qkv = np.asarray(w_qkv, dtype=np.float32)
    w_proj = np.asarray(w_proj, dtype=np.float32)
    freqs_cos = np.asarray(freqs_cos, dtype=np.float32)
    freqs_sin = np.asarray(freqs_sin, dtype=np.float32)

    aux = np.zeros((128, AUXW), np.float32)
    # interleaved-pair rope tables expanded to [HD, T]
    aux[:, COS_OFF:COS_OFF + T] = np.repeat(freqs_cos.T, 2, axis=0)
    aux[:, SIN_OFF:SIN_OFF + T] = np.repeat(freqs_sin.T, 2, axis=0)
    # signed pair-rotation matrix: rot = P.T @ t, rot[2r] = -t[2r+1], rot[2r+1] = t[2r]
    idx = np.arange(0, HD, 2)
    aux[idx + 1, ROT_OFF + idx] = -1.0
    aux[idx, ROT_OFF + idx + 1] = 1.0
    r = np.arange(128)[:, None]
    jj = np.arange(128)[None, :]
    # transposed-score orientation [k, q]: A keeps k<=q, B keeps k>q
    aux[:, MA_OFF:MA_OFF + 128] = (r <= jj)
    aux[:, MB_OFF:MB_OFF + 128] = (r > jj)
    aux[:, ID_OFF:ID_OFF + 128] = (r == jj)
    aux[:, ONE_OFF:ONE_OFF + 130] = 1.0
    aux = aux.astype(ml_dtypes.bfloat16)

    xTs = [np.ascontiguousarray(x[b].T).astype(ml_dtypes.bfloat16) for b in range(2)]
    in_maps = []
    for c in range(8):
        b, g = divmod(c, 4)
        wq = w_qkv[g * NQL * HD:(g + 1) * NQL * HD]          # [512, C]
        wk = w_qkv[NH * HD + g * HD: NH * HD + (g + 1) * HD]  # [128, C]
        wv = w_qkv[(NH + NKV) * HD + g * HD: (NH + NKV) * HD + (g + 1) * HD]
        wqkvT = np.ascontiguousarray(
            np.concatenate([wq, wk, wv], axis=0).T).astype(ml_dtypes.bfloat16)
        wpT = np.ascontiguousarray(
            w_proj[:, g * NQL * HD:(g + 1) * NQL * HD].T).astype(ml_dtypes.bfloat16)
        in_maps.append({
            "xT": xTs[b], "wqkvT": wqkvT, "wpT": wpT, "aux": aux,
        })
    return in_maps


def _run(in_maps, trace=False):
    if "nc" not in _CACHE:
        _CACHE["nc"] = _build_program()
    return run_bass_kernel_spmd(_CACHE["nc"], in_maps, core_ids=list(range(8)),
                                trace=False)


def kernel(x, w_qkv, w_proj, freqs_cos, freqs_sin, mask=None, _trace=False):
    in_maps = _prep_inputs(x, w_qkv, w_proj, freqs_cos, freqs_sin)
    res = _run(in_maps, trace=_trace)
    out = np.empty((2, T, C), dtype=np.float32)
    for b in range(2):
        acc = res.results[b * 4]["outT"].astype(np.float32)
        for g in range(1, 4):
            acc = acc + res.results[b * 4 + g]["outT"].astype(np.float32)
        out[b] = acc.T
    if _trace:
        return out, res
    return out


# revision 3
# speedup vs baseline: 3.3740x; 1.1746x over previous
"""TRN2 Bass kernel for sliding-window causal GQA attention block (v4).

Reference computation (B=2, T=2048, C=2048, NH=16, NKV=4, HD=128, WIN=512):
  qkv = x @ w_qkv.T ; RoPE(q, k) ; GQA repeat ; banded causal attention
  (keys j in [i-511, i]) ; y @ w_proj.T

Sharding: 8 cores = batch (2) x kv-head-group (4) tensor parallel.
Core c = b*4+g owns batch b, q-heads [4g..4g+4), kv head g. Each core
computes a partial output (contribution of its 512 y-dims to all 2048
out dims); host sums the 4 partials per batch.

Key design points (v4):
- All PE work in bf16 (FWL weight loads, full-rate matmuls).
- Attention scores are computed TRANSPOSED ([key, query]) per key
  tile, so exp'd probs feed the P@V matmul directly as the moving
  operand -- no PE transposes of probs.  Causal / window masks are 0/1
  multiplies on the probs (off the critical chain).
- P@V and the softmax denominator accumulate per (head, 512-token
  q chunk) with per-element has_written semantics: one matmul per
  contributing key tile, each writing just its valid column span.
- Denominators come from ones-vector matmuls; normalization of y^T is
  one reciprocal + ones-outer-product broadcast multiply per chunk.
- DMA op count is minimized (the HWDGE FIFOs serialize and each op
  carries ~2us of fixed cost): one aux-constant load, one wqkv load,
  one wproj load, 4 x-chunk loads, 4 output stores.  V tiles are
  transposed on the PE (identity matmul), not via xbar DMA.

``_build_program(reps=N)`` unrolls the whole body N times (fresh tile
pools per rep, all DMAs re-issued) -- used by the bench to measure
per-execution device time free of dispatch overhead.
"""
import sys
sys.path.insert(0, '/opt/trn_rl_repo')
import numpy as np
import ml_dtypes

import concourse.bass as bass
from concourse import bacc
import concourse.tile as tile
from concourse import mybir
from concourse.bass_utils import run_bass_kernel_spmd

T = 2048
C = 2048
HD = 128
NH = 16
NKV = 4
NQL = 4           # q heads per core
WIN = 512
QKVF = NQL * HD + 2 * HD   # 768 local qkv features
SCALE = float(1.0 / np.sqrt(HD))
QB = T // 128     # 16 q/k blocks
KC = 16           # contraction tiles (C/128)
NCH = T // 512    # 4 token chunks

# aux constant-pack column offsets
COS_OFF = 0
SIN_OFF = 2048
ROT_OFF = 4096
MA_OFF = 4224
MB_OFF = 4352
ID_OFF = 4480
ONE_OFF = 4608
AUXW = 4608 + 130

f32 = mybir.dt.float32
bf16 = mybir.dt.bfloat16

_CACHE = {}


def _emit_body(nc, tc, dram, rep):
    """One full forward pass. dram: dict of DRAM parameter APs.

    KERNEL_PHASES env (debug/bench only): "1" = qkv only, "12" = +attn,
    "123" (default) = full. Disabled phases are replaced by a liveness
    store so DCE cannot remove the remaining work.
    """
    import os as _os
    _phases = _os.environ.get("KERNEL_PHASES", "123")
    r = rep

    with tc.tile_pool(name=f"persist{r}", bufs=1) as persist, \
         tc.tile_pool(name=f"qkv_out{r}", bufs=1) as qkv_out:

        # ---- constants: one packed load on the ACT ring, wproj on the
        # ACT ring, wqkv on the gpsimd (SWDGE) ring so all three start
        # in parallel with the first x chunk on the SP ring ----
        aux = persist.tile([128, AUXW], bf16, tag="aux")
        nc.scalar.dma_start(out=aux, in_=dram["aux"][:])
        cos_sb = aux[:, COS_OFF:COS_OFF + T]
        sin_sb = aux[:, SIN_OFF:SIN_OFF + T]
        rp_sb = aux[:, ROT_OFF:ROT_OFF + 128]
        mA_sb = aux[:, MA_OFF:MA_OFF + 128]
        mB_sb = aux[:, MB_OFF:MB_OFF + 128]
        id_sb = aux[:, ID_OFF:ID_OFF + 128]
        o1_sb = aux[:, ONE_OFF:ONE_OFF + 1]
        oc_sb = aux[0:1, ONE_OFF + 1:ONE_OFF + 129]
        wp_sb = persist.tile([128, NQL, C], bf16, tag="wp")
        nc.scalar.dma_start(out=wp_sb, in_=dram["wpP"][:])
        w_sb = persist.tile([128, KC, QKVF], bf16, tag="w")
        nc.gpsimd.dma_start(out=w_sb, in_=dram["wP"][:])

        # qkv outputs (transposed layout [feature, token], bf16)
        qT = [qkv_out.tile([HD, T], bf16, tag=f"qT{h}", name=f"qT{r}_{h}")
              for h in range(NQL)]
        kT = qkv_out.tile([HD, T], bf16, tag="kT", name=f"kT{r}")
        v_sb = [qkv_out.tile([128, HD], bf16, tag=f"v{t}", name=f"v{r}_{t}")
                for t in range(QB)]
        yT = [qkv_out.tile([HD, T], bf16, tag=f"yT{h}", name=f"yT{r}_{h}")
              for h in range(NQL)]
        rr_sb = [qkv_out.tile([1, T], bf16, tag=f"rr{h}", name=f"rr{r}_{h}")
                 for h in range(NQL)]

        # ---- phase 1: QKV + rope + v-transpose, chunk by chunk ----
        with tc.tile_pool(name=f"xs{r}", bufs=2) as xsp, \
             tc.tile_pool(name=f"vtb{r}", bufs=1) as vtbp, \
             tc.tile_pool(name=f"rtp{r}", bufs=2) as rtp, \
             tc.tile_pool(name=f"qkps{r}", bufs=3, space="PSUM") as qkps, \
             tc.tile_pool(name=f"rops{r}", bufs=2, space="PSUM") as rops, \
             tc.tile_pool(name=f"vtps{r}", bufs=2, space="PSUM") as vtps:
            vTb = vtbp.tile([HD, T], bf16, tag="vTb", name=f"vTb{r}")
            xt = []
            for n in range(NCH):
                x_n = xsp.tile([128, KC, 512], bf16, tag="x", name=f"x{r}_{n}")
                nc.sync.dma_start(out=x_n, in_=dram["xP"][n])
                xt.append(x_n)

            for n in range(NCH):
                ns = slice(n * 512, (n + 1) * 512)
                # qkv for this 512-token chunk
                for m in range(QKVF // 128):
                    acc = qkps.tile([128, 512], f32, tag="acc", name=f"acc{r}_{n}_{m}")
                    for k in range(KC):
                        nc.tensor.matmul(acc, w_sb[:, k, m * 128:(m + 1) * 128],
                                         xt[n][:, k, :],
                                         start=(k == 0), stop=(k == KC - 1))
                    if m < NQL:
                        nc.vector.tensor_copy(out=qT[m][:, ns], in_=acc)
                    elif m == NQL:
                        nc.vector.tensor_copy(out=kT[:, ns], in_=acc)
                    else:
                        nc.vector.tensor_copy(out=vTb[:, ns], in_=acc)
                # rope this chunk (PE rotate via signed permutation
                # matrix; DVE cannot cross partitions)
                for th in range(NQL + 1):
                    src = qT[th] if th < NQL else kT
                    rot = rops.tile([HD, 512], f32, tag="rot", name=f"rot{r}_{n}_{th}")
                    nc.tensor.matmul(rot, rp_sb, src[:, ns], start=True, stop=True)
                    tmp = rtp.tile([HD, 512], bf16, tag="tmp")
                    nc.vector.tensor_mul(out=tmp, in0=rot, in1=sin_sb[:, ns])
                    nc.vector.tensor_mul(out=src[:, ns], in0=src[:, ns], in1=cos_sb[:, ns])
                    nc.vector.tensor_add(out=src[:, ns], in0=src[:, ns], in1=tmp)
                # v transpose on the PE (identity matmul)
                for t in range(4 * n, 4 * n + 4):
                    vt = vtps.tile([128, 128], bf16, tag="vt", name=f"vt{r}_{t}")
                    nc.tensor.transpose(vt, vTb[:, t * 128:(t + 1) * 128], id_sb)
                    nc.vector.tensor_copy(out=v_sb[t], in_=vt)

        if "2" not in _phases:
            # liveness: store q/k so phase 1 cannot be DCE'd
            for h in range(NQL):
                nc.sync.dma_start(
                    out=dram["outP"][0:1, :, h * 4:(h + 1) * 4, :].rearrange("a p m t -> p (a m t)"),
                    in_=qT[h])
            nc.sync.dma_start(
                out=dram["outP"][1:2, :, 0:4, :].rearrange("a p m t -> p (a m t)"), in_=kT)
            return

        # ---- phase 2: attention, key-tile-major transposed scores;
        # P@V + denominator accumulate per (head, 512-token q chunk),
        # consumed one chunk behind the score production ----
        probs_t = [[None] * QB for _ in range(NQL)]

        def consume_chunk(c, ypps, dnps):
            jlo = max(0, 4 * c - 4)
            jhi = 4 * c + 3
            for h in range(NQL):
                yp = ypps.tile([HD, 512], f32, tag="yp", name=f"yp{r}_{c}_{h}")
                dn = dnps.tile([1, 512], f32, tag="dn", name=f"dn{r}_{c}_{h}")
                for j in range(jlo, jhi + 1):
                    qlo = max(4 * c, j)
                    qhi = min(4 * c + 3, j + 4)
                    sl = probs_t[h][j][:, (qlo - j) * 128:(qhi - j + 1) * 128]
                    cs = slice((qlo - 4 * c) * 128, (qhi - 4 * c + 1) * 128)
                    nc.tensor.matmul(yp[:, cs], v_sb[j], sl,
                                     start=(j == jlo), stop=(j == jhi))
                    nc.tensor.matmul(dn[:, cs], o1_sb, sl,
                                     start=(j == jlo), stop=(j == jhi))
                with nc.allow_low_precision(reason="bf16 reciprocal row; 0.4% ok"):
                    nc.vector.reciprocal(rr_sb[h][:, c * 512:(c + 1) * 512], dn)
                nc.vector.tensor_copy(out=yT[h][:, c * 512:(c + 1) * 512], in_=yp)

        with tc.tile_pool(name=f"prb{r}", bufs=10) as prb, \
             tc.tile_pool(name=f"stps{r}", bufs=2, space="PSUM") as stps, \
             tc.tile_pool(name=f"ypps{r}", bufs=2, space="PSUM") as ypps, \
             tc.tile_pool(name=f"dnps{r}", bufs=2, space="PSUM") as dnps:
            for j in range(QB):
                w = 128 * min(5, QB - j)
                for h in range(NQL):
                    # scores^T for key tile j against q blocks j..j+4
                    st = stps.tile([128, 640], f32, tag="st", name=f"st{r}_{j}_{h}")
                    for c0 in range(0, w, 512):
                        cw = min(512, w - c0)
                        nc.tensor.matmul(st[:, c0:c0 + cw],
                                         kT[:, j * 128:(j + 1) * 128],
                                         qT[h][:, j * 128 + c0: j * 128 + c0 + cw],
                                         start=True, stop=True)
                    pb = prb.tile([128, 640], bf16, tag=f"pb{h}", name=f"pb{r}_{j}_{h}")
                    nc.scalar.activation(out=pb[:, :w], in_=st[:, :w],
                                         func=mybir.ActivationFunctionType.Exp,
                                         scale=SCALE)
                    # multiplicative 0/1 masks after exp: diagonal tile
                    # (cols 0:128, keep k<=q) and window-edge tile
                    # (cols 512:640, keep k>q)
                    nc.vector.tensor_mul(out=pb[:, 0:128], in0=pb[:, 0:128], in1=mA_sb)
                    if w == 640:
                        nc.vector.tensor_mul(out=pb[:, 512:640], in0=pb[:, 512:640], in1=mB_sb)
                    probs_t[h][j] = pb
                if j in (4, 8, 12):
                    consume_chunk(j // 4 - 1, ypps, dnps)
            consume_chunk(3, ypps, dnps)

        if "3" not in _phases:
            for h in range(NQL):
                nc.sync.dma_start(
                    out=dram["outP"][2:3, :, h * 4:(h + 1) * 4, :].rearrange("a p m t -> p (a m t)"),
                    in_=yT[h])
                nc.sync.dma_start(
                    out=dram["outP"][3:4, h:h + 1, 0:4, :].rearrange("a p m t -> p (a m t)"),
                    in_=rr_sb[h])
            return

        # ---- phase 3: normalize y^T per chunk + projection ----
        with tc.tile_pool(name=f"outst{r}", bufs=2) as outst, \
             tc.tile_pool(name=f"pjps{r}", bufs=3, space="PSUM") as pjps, \
             tc.tile_pool(name=f"bcps{r}", bufs=2, space="PSUM") as bcps:
            for n in range(NCH):
                ns = slice(n * 512, (n + 1) * 512)
                for h in range(NQL):
                    bc = bcps.tile([128, 512], f32, tag="bc", name=f"bc{r}_{n}_{h}")
                    nc.tensor.matmul(bc, oc_sb, rr_sb[h][:, ns], start=True, stop=True)
                    nc.vector.tensor_mul(out=yT[h][:, ns], in0=yT[h][:, ns], in1=bc)
                os_big = outst.tile([128, C // 128, 512], bf16, tag="os", name=f"os{r}_{n}")
                for mo in range(C // 128):
                    pp = pjps.tile([128, 512], f32, tag="pp", name=f"pp{r}_{mo}_{n}")
                    for kd in range(NQL):
                        nc.tensor.matmul(pp, wp_sb[:, kd, mo * 128:(mo + 1) * 128],
                                         yT[kd][:, ns],
                                         start=(kd == 0), stop=(kd == NQL - 1))
                    nc.vector.tensor_copy(out=os_big[:, mo, :], in_=pp)
                nc.sync.dma_start(out=dram["outP"][n], in_=os_big)


def _build_program(reps=1):
    nc = bacc.Bacc()
    dram = {
        "xP": nc.declare_dram_parameter("xP", [NCH, 128, KC, 512], bf16, isOutput=False),
        "wP": nc.declare_dram_parameter("wP", [128, KC, QKVF], bf16, isOutput=False),
        "wpP": nc.declare_dram_parameter("wpP", [128, NQL, C], bf16, isOutput=False),
        "aux": nc.declare_dram_parameter("aux", [128, AUXW], bf16, isOutput=False),
        "outP": nc.declare_dram_parameter("outP", [NCH, 128, C // 128, 512], bf16, isOutput=True),
    }

    import os as _os
    _tsim = _os.environ.get("KERNEL_TRACE_SIM", "0") == "1"
    with tile.TileContext(nc, trace_sim=_tsim) as tc:
        for rep in range(reps):
            _emit_body(nc, tc, dram, rep)
    nc.finalize()
    return nc


def _prep_inputs(x, w_qkv, w_proj, freqs_cos, freqs_sin):
    """Build the 8 per-core input maps (host-side shard + transpose)."""
    x = np.asarray(x, dtype=np.float32)
    w_qkv = np.asarray(w_qkv, dtype=np.float32)
    w_proj = np.asarray(w_proj, dtype=np.float32)
    freqs_cos = np.asarray(freqs_cos, dtype=np.float32)
    freqs_sin = np.asarray(freqs_sin, dtype=np.float32)

    aux = np.zeros((128, AUXW), np.float32)
    # interleaved-pair rope tables expanded to [HD, T]
    aux[:, COS_OFF:COS_OFF + T] = np.repeat(freqs_cos.T, 2, axis=0)
    aux[:, SIN_OFF:SIN_OFF + T] = np.repeat(freqs_sin.T, 2, axis=0)
    # signed pair-rotation matrix: rot = P.T @ t, rot[2r] = -t[2r+1], rot[2r+1] = t[2r]
    idx = np.arange(0, HD, 2)
    aux[idx + 1, ROT_OFF + idx] = -1.0
    aux[idx, ROT_OFF + idx + 1] = 1.0
    r = np.arange(128)[:, None]
    jj = np.arange(128)[None, :]
    # transposed-score orientation [k, q]: A keeps k<=q, B keeps k>q
    aux[:, MA_OFF:MA_OFF + 128] = (r <= jj)
    aux[:, MB_OFF:MB_OFF + 128] = (r > jj)
    aux[:, ID_OFF:ID_OFF + 128] = (r == jj)
    aux[:, ONE_OFF:ONE_OFF + 130] = 1.0
    aux = aux.astype(ml_dtypes.bfloat16)

    # pre-tiled x: xP[n, p, k, t] = x[b].T[k*128+p, n*512+t] -> every DMA contiguous
    xPs = [np.ascontiguousarray(
        x[b].T.reshape(KC, 128, NCH, 512).transpose(2, 1, 0, 3)).astype(ml_dtypes.bfloat16)
        for b in range(2)]
    in_maps = []
    for c in range(8):
        b, g = divmod(c, 4)
        wq = w_qkv[g * NQL * HD:(g + 1) * NQL * HD]          # [512, C]
        wk = w_qkv[NH * HD + g * HD: NH * HD + (g + 1) * HD]  # [128, C]
        wv = w_qkv[(NH + NKV) * HD + g * HD: (NH + NKV) * HD + (g + 1) * HD]
        wqkvT = np.concatenate([wq, wk, wv], axis=0).T  # [C, QKVF]
        wP = np.ascontiguousarray(
            wqkvT.reshape(KC, 128, QKVF).transpose(1, 0, 2)).astype(ml_dtypes.bfloat16)
        wpT = w_proj[:, g * NQL * HD:(g + 1) * NQL * HD].T  # [512, C]
        wpP = np.ascontiguousarray(
            wpT.reshape(NQL, 128, C).transpose(1, 0, 2)).astype(ml_dtypes.bfloat16)
        in_maps.append({
            "xP": xPs[b], "wP": wP, "wpP": wpP, "aux": aux,
        })
    return in_maps


def _run(in_maps, trace=False):
    if "nc" not in _CACHE:
        _CACHE["nc"] = _build_program()
    return run_bass_kernel_spmd(_CACHE["nc"], in_maps, core_ids=list(range(8)),
                                trace=False)


def kernel(x, w_qkv, w_proj, freqs_cos, freqs_sin, mask=None, _trace=False):
    in_maps = _prep_inputs(x, w_qkv, w_proj, freqs_cos, freqs_sin)
    res = _run(in_maps, trace=_trace)
    out = np.empty((2, T, C), dtype=np.float32)
    for b in range(2):
        acc = res.results[b * 4]["outP"].astype(np.float32)
        for g in range(1, 4):
            acc = acc + res.results[b * 4 + g]["outP"].astype(np.float32)
        # outP[n, p, mo, t] = out^T[mo*128+p, n*512+t]
        outT = acc.transpose(2, 1, 0, 3).reshape(C, T)
        out[b] = outT.T
    if _trace:
        return out, res
    return out
